# revision 21
# baseline (speedup 1.0000x reference)
"""Multi-head causal attention (B=4, S=2048, DM=1024, H=16) on 8 TRN2 cores.

Sharding: core = 2*b + hg  (b = batch 0..3, hg = head-group 0..1, 8 heads each).
Each core computes, for its batch and its 8 heads:
    Q^T = (Wq_hg)^T x^T, K^T likewise   [512, 2048]  (d-major)
    V   = x Wv_hg                       [2048, 512]  (s-major, per-head 65-col
                                                      blocks with a ones column
                                                      for the softmax row-sums)
    S^T(kt) = K_kt Q^T / masked-exp -> A^T tiles  [128k, q]
    O^T_unnorm[65, q] = sum_kt V_aug(kt)^T-style matmul (lhsT=V_aug, rhs=A^T)
        row 63/64 = row-sums; normalized via reciprocal + DMA partition
        broadcast; result -> O^T [512, 2048] bf16
    out_part = O^T-contracted projection with Wo rows of this head group
Host sums the two head-group partials per batch and adds bo.

All matmul inputs bf16 (fp32 PSUM accumulation). Host transposes x and
converts dtypes, so no on-device transposes are needed anywhere.
"""

import sys

for _p in ("/opt/trn_rl_repo",):
    if _p not in sys.path:
        sys.path.insert(0, _p)

import numpy as np
import ml_dtypes

import concourse.bass as bass
import concourse.mybir as mybir
import concourse.tile as tile
from concourse import bacc
from concourse.bass_utils import run_bass_kernel_spmd

B, S, DM, H, DK = 4, 2048, 1024, 16, 64
HPC = 8          # heads per core
DQK = 512        # q/k/v width per core
NPAIR = 4        # head pairs per core (one per 128-row d-tile)
SC = S // 512    # 512-wide q/s chunks
KT = S // 128    # 128-wide key tiles
A_BUFS = 8       # live bf16 A^T tiles (diagonal chunk only: 4 + pipeline)

BF16 = mybir.dt.bfloat16
F32 = mybir.dt.float32
F8 = mybir.dt.float8e4
BF = ml_dtypes.bfloat16
EXP = mybir.ActivationFunctionType.Exp
MUL = mybir.AluOpType.mult
DR = mybir.MatmulPerfMode.DoubleRow
# A = exp(s/8 - EXPOFF): keeps unnormalized fp8e4m3 softmax weights well
# inside [0, 240] (max causal logit ~6); the e^-EXPOFF factor cancels in
# the rowsum normalization.
EXPOFF = 2.0

LAST_RESULT = None


def _pbcast(row_ap, nparts):
    """Broadcast a [1, N] DRAM AP along the partition axis -> [nparts, N]."""
    return bass.AP(
        tensor=row_ap.tensor,
        offset=row_ap.offset,
        ap=[[0, nparts]] + list(row_ap.ap)[1:],
    )


def _reshape128(row_ap, width):
    """View a [1, 128*width] DRAM AP as [128, width]."""
    return bass.AP(
        tensor=row_ap.tensor,
        offset=row_ap.offset,
        ap=[[width, 128], [1, width]],
    )


def _emit(nc, nkt, schedule="staged"):
    """Emit the whole per-core kernel. nkt = contraction tiles (8, or 9 when
    biases are folded in via an augmented ones-row in xT)."""
    xT = nc.dram_tensor("xT", [nkt * 128, S], BF16, kind="ExternalInput").ap()
    wq = nc.dram_tensor("wq", [nkt * 128, DQK], BF16, kind="ExternalInput").ap()
    wk = nc.dram_tensor("wk", [nkt * 128, DQK], BF16, kind="ExternalInput").ap()
    wv = nc.dram_tensor("wv", [nkt * 128, DQK], BF16, kind="ExternalInput").ap()
    wo = nc.dram_tensor("wo", [DQK, DM], BF16, kind="ExternalInput").ap()
    msk = nc.dram_tensor("mask", [128, 128], BF16, kind="ExternalInput").ap()
    out = nc.dram_tensor("out", [S, DM], BF16, kind="ExternalOutput").ap()

    xT3 = xT.rearrange("(t p) s -> p t s", p=128)
    wq3 = wq.rearrange("(t p) d -> p t d", p=128)
    wk3 = wk.rearrange("(t p) d -> p t d", p=128)
    wv3 = wv.rearrange("(t p) d -> p t d", p=128)
    wo3 = wo.rearrange("(t p) m -> p t m", p=128)

    with tile.TileContext(nc) as tc:
        with (
            tc.tile_pool(name="big", bufs=1) as big,
            tc.tile_pool(name="wqkv", bufs=1) as wp,
            tc.tile_pool(name="xc", bufs=2) as xcp,
            tc.tile_pool(name="ap", bufs=A_BUFS) as apool,
            tc.tile_pool(name="a8", bufs=8) as a8p,
            tc.tile_pool(name="rcp", bufs=5) as rcpp,
            tc.tile_pool(name="bc", bufs=4) as bcp,
            tc.tile_pool(name="ob", bufs=2) as obp,
            tc.tile_pool(name="mm", bufs=2, space="PSUM") as mmp,
            tc.tile_pool(name="sc", bufs=2, space="PSUM") as scp,
            tc.tile_pool(name="otp", bufs=2, space="PSUM") as otpp,
            tc.tile_pool(name="rb", bufs=8, space="DRAM") as rbp,
        ):
            # Q^T in two variants with the other pair-half's rows zeroed, so
            # scores matmuls use full K=128 contraction at partition base 0
            # (concurrent disjoint-row-group matmuls fault on this setup).
            qt_e = big.tile([128, NPAIR, S], BF16, tag="qte")
            qt_o = big.tile([128, NPAIR, S], BF16, tag="qto")
            kt_ = big.tile([128, NPAIR, S], BF16, tag="kt")
            v = big.tile([128, KT, HPC * 65], BF16, tag="v")
            ot = big.tile([128, NPAIR, S], BF16, tag="ot")
            wo_sb = big.tile([128, NPAIR, DM], BF16, tag="wo")
            mask_sb = big.tile([128, 128], BF16, tag="mask")

            # zero fills off the Vector engine (Pool is otherwise idle);
            # the V ones-fill only needs column 64 of each 65-wide block
            nc.gpsimd.memset(qt_e, 0.0)
            nc.gpsimd.memset(qt_o, 0.0)
            v_ones = v.rearrange("p kt (h c) -> p kt h c", c=65)[:, :, :, 64:65]
            nc.gpsimd.memset(v_ones, 1.0)
            # fp8 V blocks are 80 wide (vs 65): dual-fp8 weight loads
            # need 16-aligned column offsets/counts; col 64 is the ones
            # column, cols 65-79 are zero padding
            v8 = big.tile([128, KT, HPC * 80], F8, tag="v8")
            v8_pad = v8.rearrange("p kt (h c) -> p kt h c", c=80)[:, :, :, 64:80]
            nc.gpsimd.memset(v8_pad, 0.0)
            v8_ones = v8.rearrange("p kt (h c) -> p kt h c", c=80)[:, :, :, 64:65]
            nc.gpsimd.memset(v8_ones, 1.0)
            expoff_sb = big.tile([128, 1], F32, tag="expoff")
            nc.gpsimd.memset(expoff_sb, -EXPOFF)

            wq_sb = wp.tile([128, nkt, DQK], BF16, tag="wq")
            wk_sb = wp.tile([128, nkt, DQK], BF16, tag="wk")
            wv_sb = wp.tile([128, nkt, DQK], BF16, tag="wv")

            def qkv_dma(st):
                xc = xcp.tile([128, nkt, 512], BF16, tag="xc")
                for k in range(nkt):
                    nc.sync.dma_start(
                        out=xc[:, k, :],
                        in_=xT3[:, k, st * 512 : (st + 1) * 512],
                    )
                return xc

            # issue stage-0 operand DMAs first so the first matmul can start
            # early; wo/mask are only needed much later
            xc0 = qkv_dma(0)
            for k in range(nkt):
                nc.sync.dma_start(out=wq_sb[:, k, :], in_=wq3[:, k, :])
            for k in range(nkt):
                nc.sync.dma_start(out=wk_sb[:, k, :], in_=wk3[:, k, :])
            for k in range(nkt):
                nc.sync.dma_start(out=wv_sb[:, k, :], in_=wv3[:, k, :])
            nc.sync.dma_start(out=mask_sb, in_=msk)
            for dt_i in range(NPAIR):
                nc.sync.dma_start(out=wo_sb[:, dt_i, :], in_=wo3[:, dt_i, :])

            def qkv_stile(st, xc=None):
                if xc is None:
                    xc = qkv_dma(st)
                ss = slice(st * 512, (st + 1) * 512)
                for w_sb, dst in ((wq_sb, None), (wk_sb, kt_)):
                    for dt_i in range(NPAIR):
                        ps = mmp.tile([128, 512], F32, tag="mm")
                        for k in range(nkt):
                            nc.tensor.matmul(
                                out=ps,
                                lhsT=w_sb[:, k, dt_i * 128 : (dt_i + 1) * 128],
                                rhs=xc[:, k, :],
                                start=(k == 0),
                                stop=(k == nkt - 1),
                            )
                        if dst is None:
                            nc.vector.tensor_copy(
                                out=qt_e[0:64, dt_i, ss], in_=ps[0:64, :]
                            )
                            nc.vector.tensor_copy(
                                out=qt_o[64:128, dt_i, ss], in_=ps[64:128, :]
                            )
                        else:
                            nc.vector.tensor_copy(out=dst[:, dt_i, ss], in_=ps)
                for ssub in range(4):
                    ps = mmp.tile([128, 512], F32, tag="mm")
                    for k in range(nkt):
                        nc.tensor.matmul(
                            out=ps,
                            lhsT=xc[:, k, ssub * 128 : (ssub + 1) * 128],
                            rhs=wv_sb[:, k, :],
                            start=(k == 0),
                            stop=(k == nkt - 1),
                        )
                    kti = st * 4 + ssub
                    ps4 = ps.rearrange("p (h d) -> p h d", d=DK)
                    v4 = v[:, kti, :].rearrange("p (h c) -> p h c", c=65)
                    nc.vector.tensor_copy(out=v4[:, :, 0:DK], in_=ps4)
                    v84 = v8[:, kti, :].rearrange("p (h c) -> p h c", c=80)
                    nc.vector.tensor_copy(out=v84[:, :, 0:DK], in_=ps4)

            def attn(pr, c_lo, c_hi):
                """Attention for head pair pr, q chunks c_lo..c_hi.

                Off-diagonal (fully causal-valid) key tiles store A in fp8
                pairs and contract with fp8 V via DoubleRow matmuls (2 key
                tiles per pass); the diagonal chunk stays bf16, which also
                keeps the few-key early rows at bf16 precision."""
                a_tiles = {}
                a8_tiles = {}
                for kti in range(4 * c_hi + 4):
                    cs0 = kti // 4
                    for c in range(max(cs0, c_lo), c_hi + 1):
                        qoff = 128 * (kti % 4) if c == cs0 else 0
                        ps = scp.tile([128, 2, 512], F32, tag="sc")
                        for hh in range(2):
                            qsrc = qt_e if hh == 0 else qt_o
                            nc.tensor.matmul(
                                out=ps[:, hh, qoff:512],
                                lhsT=kt_[:, pr, kti * 128 : (kti + 1) * 128],
                                rhs=qsrc[:, pr, c * 512 + qoff : (c + 1) * 512],
                                start=True,
                                stop=True,
                            )
                        if c == cs0:
                            at = apool.tile([128, 2, 512], BF16, tag="a")
                            nc.scalar.activation(
                                out=at[:, :, qoff:512],
                                in_=ps[:, :, qoff:512],
                                func=EXP,
                                scale=0.125,
                                bias=expoff_sb,
                            )
                            dg = at[:, :, qoff : qoff + 128]
                            nc.vector.tensor_tensor(
                                out=dg, in0=dg, in1=_pbcast2(mask_sb), op=MUL
                            )
                            a_tiles[(kti, c)] = at
                        else:
                            key = (kti // 2, c)
                            if key not in a8_tiles:
                                a8t = a8p.tile([128, 2, 2, 512], F8, tag="a8")
                                a8_tiles[key] = a8t
                            nc.scalar.activation(
                                out=a8_tiles[key][:, kti % 2, :, :],
                                in_=ps,
                                func=EXP,
                                scale=0.125,
                                bias=expoff_sb,
                            )
                    C = kti // 4
                    if kti % 4 == 3 and C >= c_lo:
                        for hh in range(2):
                            h = pr * 2 + hh
                            ops = otpp.tile([128, 512], F32, tag="otp")
                            for t2 in range(2 * C):
                                nc.tensor.matmul(
                                    out=ops[0:80, :],
                                    lhsT=v8[
                                        :, 2 * t2 : 2 * t2 + 2,
                                        h * 80 : (h + 1) * 80,
                                    ],
                                    rhs=a8_tiles[(t2, C)][:, :, hh, :],
                                    start=(t2 == 0),
                                    stop=False,
                                    perf_mode=DR,
                                )
                            for j in range(4):
                                k2 = 4 * C + j
                                qoff = 128 * j
                                nc.tensor.matmul(
                                    out=ops[0:65, qoff:512],
                                    lhsT=v[:, k2, h * 65 : (h + 1) * 65],
                                    rhs=a_tiles[(k2, C)][:, hh, qoff:512],
                                    start=(C == 0 and j == 0),
                                    stop=(j == 3),
                                )
                            # normalize: reciprocal of the rowsum row in
                            # place (single-partition DVE op), partition
                            # broadcast via one DRAM bounce; the final
                            # multiplies run on Pool so the Vector stream
                            # never stalls waiting for the DMA round trip
                            rs = rcpp.tile([128, 512], F32, tag="rs")
                            nc.vector.tensor_copy(
                                out=rs[0:65, :], in_=ops[0:65, :]
                            )
                            rcp = rcpp.tile([1, 512], F32, tag="rcp")
                            nc.vector.reciprocal(out=rcp, in_=rs[64:65, :])
                            rd2 = rbp.tile([1, 512], F32, tag="rb2")
                            nc.sync.dma_start(out=rd2, in_=rcp)
                            bc = bcp.tile([128, 512], F32, tag="bc")
                            nc.sync.dma_start(
                                out=bc[0:64, :], in_=_pbcast(rd2, 64)
                            )
                            csl = slice(C * 512, (C + 1) * 512)
                            if hh == 0:
                                nc.gpsimd.tensor_tensor(
                                    out=ot[0:64, pr, csl],
                                    in0=rs[0:64, :],
                                    in1=bc[0:64, :],
                                    op=MUL,
                                )
                            else:
                                tmp = bcp.tile([64, 512], BF16, tag="tmp")
                                nc.gpsimd.tensor_tensor(
                                    out=tmp,
                                    in0=rs[0:64, :],
                                    in1=bc[0:64, :],
                                    op=MUL,
                                )
                                nc.sync.dma_start(
                                    out=ot[64:128, pr, csl], in_=tmp
                                )

            def proj(chunk):
                for ssub in range(4):
                    srow = chunk * 4 + ssub
                    for mc in range(2):
                        ps = mmp.tile([128, 512], F32, tag="mm")
                        for dt_i in range(NPAIR):
                            nc.tensor.matmul(
                                out=ps,
                                lhsT=ot[:, dt_i, srow * 128 : (srow + 1) * 128],
                                rhs=wo_sb[:, dt_i, mc * 512 : (mc + 1) * 512],
                                start=(dt_i == 0),
                                stop=(dt_i == NPAIR - 1),
                            )
                        ob = obp.tile([128, 512], BF16, tag="ob")
                        nc.vector.tensor_copy(out=ob, in_=ps)
                        # bulk output rides the Activation HWDGE queue so it
                        # cannot back up the latency-critical small DMAs
                        # (normalization round-trips, input prefetch) on the
                        # sync queue
                        nc.scalar.dma_start(
                            out=out[
                                srow * 128 : (srow + 1) * 128,
                                mc * 512 : (mc + 1) * 512,
                            ],
                            in_=ob,
                        )

            if schedule == "staged":
                for st in range(SC):
                    qkv_stile(st, xc0 if st == 0 else None)
                    # proj of the previous chunk fills PE idle while the
                    # vector engine produces this stage's q/k/v copies
                    if st > 0:
                        proj(st - 1)
                    for pr in range(NPAIR):
                        attn(pr, st, st)
                proj(SC - 1)
            else:
                raise ValueError(schedule)
    return nc


def _pbcast2(mask_sb):
    """mask [128, 128] -> [128, 2, 128] with broadcast middle dim."""
    m = mask_sb[:, :]
    ap = list(m.ap)
    return bass.AP(tensor=m.tensor, offset=m.offset, ap=[ap[0], [0, 2], ap[1]])


_NC_CACHE = {}


def _get_nc(nkt):
    if nkt not in _NC_CACHE:
        nc = bacc.Bacc(
            "TRN2",
            target_bir_lowering=False,
            debug=False,
            enable_asserts=False,
            num_devices=8,
        )
        _emit(nc, nkt)
        nc.compile()
        _NC_CACHE[nkt] = nc
    return _NC_CACHE[nkt]


def kernel(**inputs):
    x = np.asarray(inputs["x"], dtype=np.float32)
    mask = np.asarray(inputs["mask"]).reshape(S, S)
    Wq = np.asarray(inputs["Wq"], dtype=np.float32)
    bq = np.asarray(inputs["bq"], dtype=np.float32)
    Wk = np.asarray(inputs["Wk"], dtype=np.float32)
    bk = np.asarray(inputs["bk"], dtype=np.float32)
    Wv = np.asarray(inputs["Wv"], dtype=np.float32)
    bv = np.asarray(inputs["bv"], dtype=np.float32)
    Wo = np.asarray(inputs["Wo"], dtype=np.float32)
    bo = np.asarray(inputs["bo"], dtype=np.float32)

    assert np.array_equal(
        mask, np.tril(np.ones((S, S), dtype=bool))
    ), "kernel specialized for the causal (tril) mask"

    bias_zero = not (bq.any() or bk.any() or bv.any())
    nkt = 8 if bias_zero else 9
    nc = _get_nc(nkt)

    # local diag-block mask in (k, q) layout: valid when q >= k
    mtile = np.triu(np.ones((128, 128), dtype=np.float32)).astype(BF)

    def aug(w, b):
        if bias_zero:
            return w.astype(BF)
        pad = np.zeros((128, w.shape[1]), dtype=np.float32)
        pad[0] = b
        return np.vstack([w, pad]).astype(BF)

    in_maps = []
    for core in range(8):
        b, hg = divmod(core, 2)
        cols = slice(hg * DQK, (hg + 1) * DQK)
        xT = x[b].T
        if not bias_zero:
            pad = np.zeros((128, S), dtype=np.float32)
            pad[0] = 1.0
            xT = np.vstack([xT, pad])
        in_maps.append(
            {
                "xT": np.ascontiguousarray(xT).astype(BF),
                "wq": aug(Wq[:, cols], bq[cols]),
                "wk": aug(Wk[:, cols], bk[cols]),
                "wv": aug(Wv[:, cols], bv[cols]),
                "wo": np.ascontiguousarray(Wo[cols, :]).astype(BF),
                "mask": mtile,
            }
        )

    res = run_bass_kernel_spmd(nc, in_maps, core_ids=list(range(8)))
    global LAST_RESULT
    LAST_RESULT = res
    parts = [np.asarray(r["out"], dtype=np.float32) for r in res.results]
    out = np.stack(
        [parts[2 * b_] + parts[2 * b_ + 1] for b_ in range(B)]
    ) + bo.astype(np.float32)
    return out.astype(np.float32)



# revision 24
# speedup vs baseline: 1.4509x; 1.4509x over previous
"""Multi-head causal attention (B=4, S=2048, DM=1024, H=16) on 8 TRN2 cores.

Sharding: core = 2*b + hg  (b = batch 0..3, hg = head-group 0..1, 8 heads each).
Each core computes, for its batch and its 8 heads:
    Q^T = (Wq_hg)^T x^T, K^T likewise   [512, 2048]  (d-major)
    V   = x Wv_hg                       [2048, 512]  (s-major, per-head 65-col
                                                      blocks with a ones column
                                                      for the softmax row-sums)
    S^T(kt) = K_kt Q^T / masked-exp -> A^T tiles  [128k, q]
    O^T_unnorm[65, q] = sum_kt V_aug(kt)^T-style matmul (lhsT=V_aug, rhs=A^T)
        row 63/64 = row-sums; normalized via reciprocal + DMA partition
        broadcast; result -> O^T [512, 2048] bf16
    out_part = O^T-contracted projection with Wo rows of this head group
Host sums the two head-group partials per batch and adds bo.

All matmul inputs bf16 (fp32 PSUM accumulation). Host transposes x and
converts dtypes, so no on-device transposes are needed anywhere.
"""

import sys

for _p in ("/opt/trn_rl_repo",):
    if _p not in sys.path:
        sys.path.insert(0, _p)

import numpy as np
import ml_dtypes

import concourse.bass as bass
import concourse.mybir as mybir
import concourse.tile as tile
from concourse import bacc
from concourse.bass_utils import run_bass_kernel_spmd

B, S, DM, H, DK = 4, 2048, 1024, 16, 64
HPC = 8          # heads per core
DQK = 512        # q/k/v width per core
NPAIR = 4        # head pairs per core (one per 128-row d-tile)
SC = S // 512    # 512-wide q/s chunks
KT = S // 128    # 128-wide key tiles
A_BUFS = 8       # live bf16 A^T tiles (diagonal chunk only: 4 + pipeline)

BF16 = mybir.dt.bfloat16
F32 = mybir.dt.float32
F8 = mybir.dt.float8e4
BF = ml_dtypes.bfloat16
EXP = mybir.ActivationFunctionType.Exp
MUL = mybir.AluOpType.mult
DR = mybir.MatmulPerfMode.DoubleRow
# A = exp(s/8 - EXPOFF): keeps unnormalized fp8e4m3 softmax weights well
# inside [0, 240] (max causal logit ~6); the e^-EXPOFF factor cancels in
# the rowsum normalization.
EXPOFF = 2.0

LAST_RESULT = None


def _pbcast(row_ap, nparts):
    """Broadcast a [1, N] DRAM AP along the partition axis -> [nparts, N]."""
    return bass.AP(
        tensor=row_ap.tensor,
        offset=row_ap.offset,
        ap=[[0, nparts]] + list(row_ap.ap)[1:],
    )


def _reshape128(row_ap, width):
    """View a [1, 128*width] DRAM AP as [128, width]."""
    return bass.AP(
        tensor=row_ap.tensor,
        offset=row_ap.offset,
        ap=[[width, 128], [1, width]],
    )


def _emit(nc, nkt, schedule="staged"):
    """Emit the whole per-core kernel. nkt = contraction tiles (8, or 9 when
    biases are folded in via an augmented ones-row in xT)."""
    xT = nc.dram_tensor("xT", [nkt * 128, S], BF16, kind="ExternalInput").ap()
    wq = nc.dram_tensor("wq", [nkt * 128, DQK], BF16, kind="ExternalInput").ap()
    wk = nc.dram_tensor("wk", [nkt * 128, DQK], BF16, kind="ExternalInput").ap()
    wv = nc.dram_tensor("wv", [nkt * 128, DQK], BF16, kind="ExternalInput").ap()
    wo = nc.dram_tensor("wo", [DQK, DM], BF16, kind="ExternalInput").ap()
    msk = nc.dram_tensor("mask", [128, 128], BF16, kind="ExternalInput").ap()
    out = nc.dram_tensor("out", [S, DM], BF16, kind="ExternalOutput").ap()

    xT3 = xT.rearrange("(t p) s -> p t s", p=128)
    wq3 = wq.rearrange("(t p) d -> p t d", p=128)
    wk3 = wk.rearrange("(t p) d -> p t d", p=128)
    wv3 = wv.rearrange("(t p) d -> p t d", p=128)
    wo3 = wo.rearrange("(t p) m -> p t m", p=128)

    with tile.TileContext(nc) as tc:
        with (
            tc.tile_pool(name="big", bufs=1) as big,
            tc.tile_pool(name="wqkv", bufs=1) as wp,
            tc.tile_pool(name="xc", bufs=2) as xcp,
            tc.tile_pool(name="ap", bufs=A_BUFS) as apool,
            tc.tile_pool(name="a8", bufs=8) as a8p,
            tc.tile_pool(name="rcp", bufs=5) as rcpp,
            tc.tile_pool(name="bc", bufs=4) as bcp,
            tc.tile_pool(name="ob", bufs=2) as obp,
            tc.tile_pool(name="mm", bufs=2, space="PSUM") as mmp,
            tc.tile_pool(name="sc", bufs=2, space="PSUM") as scp,
            tc.tile_pool(name="otp", bufs=2, space="PSUM") as otpp,
            tc.tile_pool(name="rb", bufs=8, space="DRAM") as rbp,
        ):
            # Q^T in two variants with the other pair-half's rows zeroed, so
            # scores matmuls use full K=128 contraction at partition base 0
            # (concurrent disjoint-row-group matmuls fault on this setup).
            qt_e = big.tile([128, NPAIR, S], BF16, tag="qte")
            qt_o = big.tile([128, NPAIR, S], BF16, tag="qto")
            kt_ = big.tile([128, NPAIR, S], BF16, tag="kt")
            v = big.tile([128, KT, HPC * 65], BF16, tag="v")
            ot = big.tile([128, NPAIR, S], BF16, tag="ot")
            wo_sb = big.tile([128, NPAIR, DM], BF16, tag="wo")
            mask_sb = big.tile([128, 128], BF16, tag="mask")

            # zero fills off the Vector engine (Pool is otherwise idle);
            # the V ones-fill only needs column 64 of each 65-wide block
            nc.gpsimd.memset(qt_e, 0.0)
            nc.gpsimd.memset(qt_o, 0.0)
            v_ones = v.rearrange("p kt (h c) -> p kt h c", c=65)[:, :, :, 64:65]
            nc.gpsimd.memset(v_ones, 1.0)
            # fp8 V blocks are 80 wide (vs 65): dual-fp8 weight loads
            # need 16-aligned column offsets/counts; col 64 is the ones
            # column, cols 65-79 are zero padding
            v8 = big.tile([128, KT, HPC * 80], F8, tag="v8")
            v8_pad = v8.rearrange("p kt (h c) -> p kt h c", c=80)[:, :, :, 64:80]
            nc.gpsimd.memset(v8_pad, 0.0)
            v8_ones = v8.rearrange("p kt (h c) -> p kt h c", c=80)[:, :, :, 64:65]
            nc.gpsimd.memset(v8_ones, 1.0)
            expoff_sb = big.tile([128, 1], F32, tag="expoff")
            nc.gpsimd.memset(expoff_sb, -EXPOFF)

            wq_sb = wp.tile([128, nkt, DQK], BF16, tag="wq")
            wk_sb = wp.tile([128, nkt, DQK], BF16, tag="wk")
            wv_sb = wp.tile([128, nkt, DQK], BF16, tag="wv")

            def qkv_dma(st):
                xc = xcp.tile([128, nkt, 512], BF16, tag="xc")
                for k in range(nkt):
                    nc.sync.dma_start(
                        out=xc[:, k, :],
                        in_=xT3[:, k, st * 512 : (st + 1) * 512],
                    )
                return xc

            # issue stage-0 operand DMAs first so the first matmul can start
            # early; wo/mask are only needed much later
            xc0 = qkv_dma(0)
            for k in range(nkt):
                nc.sync.dma_start(out=wq_sb[:, k, :], in_=wq3[:, k, :])
            for k in range(nkt):
                nc.sync.dma_start(out=wk_sb[:, k, :], in_=wk3[:, k, :])
            for k in range(nkt):
                nc.sync.dma_start(out=wv_sb[:, k, :], in_=wv3[:, k, :])
            nc.sync.dma_start(out=mask_sb, in_=msk)
            for dt_i in range(NPAIR):
                nc.sync.dma_start(out=wo_sb[:, dt_i, :], in_=wo3[:, dt_i, :])

            def qkv_stile(st, xc=None):
                if xc is None:
                    xc = qkv_dma(st)
                ss = slice(st * 512, (st + 1) * 512)
                for w_sb, dst in ((wq_sb, None), (wk_sb, kt_)):
                    for dt_i in range(NPAIR):
                        ps = mmp.tile([128, 512], F32, tag="mm")
                        for k in range(nkt):
                            nc.tensor.matmul(
                                out=ps,
                                lhsT=w_sb[:, k, dt_i * 128 : (dt_i + 1) * 128],
                                rhs=xc[:, k, :],
                                start=(k == 0),
                                stop=(k == nkt - 1),
                            )
                        if dst is None:
                            nc.vector.tensor_copy(
                                out=qt_e[0:64, dt_i, ss], in_=ps[0:64, :]
                            )
                            nc.vector.tensor_copy(
                                out=qt_o[64:128, dt_i, ss], in_=ps[64:128, :]
                            )
                        else:
                            nc.vector.tensor_copy(out=dst[:, dt_i, ss], in_=ps)
                for ssub in range(4):
                    ps = mmp.tile([128, 512], F32, tag="mm")
                    for k in range(nkt):
                        nc.tensor.matmul(
                            out=ps,
                            lhsT=xc[:, k, ssub * 128 : (ssub + 1) * 128],
                            rhs=wv_sb[:, k, :],
                            start=(k == 0),
                            stop=(k == nkt - 1),
                        )
                    kti = st * 4 + ssub
                    ps4 = ps.rearrange("p (h d) -> p h d", d=DK)
                    v4 = v[:, kti, :].rearrange("p (h c) -> p h c", c=65)
                    nc.vector.tensor_copy(out=v4[:, :, 0:DK], in_=ps4)
                    v84 = v8[:, kti, :].rearrange("p (h c) -> p h c", c=80)
                    nc.vector.tensor_copy(out=v84[:, :, 0:DK], in_=ps4)

            def attn(pr, c_lo, c_hi):
                """Attention for head pair pr, q chunks c_lo..c_hi.

                Off-diagonal (fully causal-valid) key tiles store A in fp8
                pairs and contract with fp8 V via DoubleRow matmuls (2 key
                tiles per pass); the diagonal chunk stays bf16, which also
                keeps the few-key early rows at bf16 precision."""
                a_tiles = {}
                a8_tiles = {}
                for kti in range(4 * c_hi + 4):
                    cs0 = kti // 4
                    for c in range(max(cs0, c_lo), c_hi + 1):
                        qoff = 128 * (kti % 4) if c == cs0 else 0
                        ps = scp.tile([128, 2, 512], F32, tag="sc")
                        for hh in range(2):
                            qsrc = qt_e if hh == 0 else qt_o
                            nc.tensor.matmul(
                                out=ps[:, hh, qoff:512],
                                lhsT=kt_[:, pr, kti * 128 : (kti + 1) * 128],
                                rhs=qsrc[:, pr, c * 512 + qoff : (c + 1) * 512],
                                start=True,
                                stop=True,
                            )
                        if c == cs0:
                            at = apool.tile([128, 2, 512], BF16, tag="a")
                            nc.scalar.activation(
                                out=at[:, :, qoff:512],
                                in_=ps[:, :, qoff:512],
                                func=EXP,
                                scale=0.125,
                                bias=expoff_sb,
                            )
                            dg = at[:, :, qoff : qoff + 128]
                            nc.vector.tensor_tensor(
                                out=dg, in0=dg, in1=_pbcast2(mask_sb), op=MUL
                            )
                            a_tiles[(kti, c)] = at
                        else:
                            key = (kti // 2, c)
                            if key not in a8_tiles:
                                a8t = a8p.tile([128, 2, 2, 512], F8, tag="a8")
                                a8_tiles[key] = a8t
                            nc.scalar.activation(
                                out=a8_tiles[key][:, kti % 2, :, :],
                                in_=ps,
                                func=EXP,
                                scale=0.125,
                                bias=expoff_sb,
                            )
                    C = kti // 4
                    if kti % 4 == 3 and C >= c_lo:
                        for hh in range(2):
                            h = pr * 2 + hh
                            ops = otpp.tile([128, 512], F32, tag="otp")
                            for t2 in range(2 * C):
                                nc.tensor.matmul(
                                    out=ops[0:80, :],
                                    lhsT=v8[
                                        :, 2 * t2 : 2 * t2 + 2,
                                        h * 80 : (h + 1) * 80,
                                    ],
                                    rhs=a8_tiles[(t2, C)][:, :, hh, :],
                                    start=(t2 == 0),
                                    stop=False,
                                    perf_mode=DR,
                                )
                            for j in range(4):
                                k2 = 4 * C + j
                                qoff = 128 * j
                                nc.tensor.matmul(
                                    out=ops[0:65, qoff:512],
                                    lhsT=v[:, k2, h * 65 : (h + 1) * 65],
                                    rhs=a_tiles[(k2, C)][:, hh, qoff:512],
                                    start=(C == 0 and j == 0),
                                    stop=(j == 3),
                                )
                            # normalize: rowsum row reshaped to [128, 4]
                            # with a direct SBUF->SBUF scatter DMA, cheap
                            # [128,4] reciprocal, then DMA partition
                            # broadcast via a DRAM bounce
                            rs = rcpp.tile([128, 512], F32, tag="rs")
                            nc.vector.tensor_copy(
                                out=rs[0:65, :], in_=ops[0:65, :]
                            )
                            rd = rbp.tile([1, 512], F32, tag="rb")
                            nc.sync.dma_start(out=rd, in_=rs[64:65, :])
                            r2 = rcpp.tile([128, 4], F32, tag="r2")
                            nc.sync.dma_start(out=r2, in_=_reshape128(rd, 4))
                            r2b = rcpp.tile([128, 4], F32, tag="r2b")
                            nc.vector.reciprocal(out=r2b, in_=r2)
                            rd2 = rbp.tile([1, 512], F32, tag="rb2")
                            nc.sync.dma_start(out=_reshape128(rd2, 4), in_=r2b)
                            bc = bcp.tile([128, 512], F32, tag="bc")
                            nc.sync.dma_start(
                                out=bc[0:64, :], in_=_pbcast(rd2, 64)
                            )
                            csl = slice(C * 512, (C + 1) * 512)
                            if hh == 0:
                                nc.vector.tensor_tensor(
                                    out=ot[0:64, pr, csl],
                                    in0=rs[0:64, :],
                                    in1=bc[0:64, :],
                                    op=MUL,
                                )
                            else:
                                tmp = bcp.tile([64, 512], BF16, tag="tmp")
                                nc.vector.tensor_tensor(
                                    out=tmp,
                                    in0=rs[0:64, :],
                                    in1=bc[0:64, :],
                                    op=MUL,
                                )
                                nc.sync.dma_start(
                                    out=ot[64:128, pr, csl], in_=tmp
                                )

            def proj(chunk):
                for ssub in range(4):
                    srow = chunk * 4 + ssub
                    for mc in range(2):
                        ps = mmp.tile([128, 512], F32, tag="mm")
                        for dt_i in range(NPAIR):
                            nc.tensor.matmul(
                                out=ps,
                                lhsT=ot[:, dt_i, srow * 128 : (srow + 1) * 128],
                                rhs=wo_sb[:, dt_i, mc * 512 : (mc + 1) * 512],
                                start=(dt_i == 0),
                                stop=(dt_i == NPAIR - 1),
                            )
                        ob = obp.tile([128, 512], BF16, tag="ob")
                        nc.vector.tensor_copy(out=ob, in_=ps)
                        nc.sync.dma_start(
                            out=out[
                                srow * 128 : (srow + 1) * 128,
                                mc * 512 : (mc + 1) * 512,
                            ],
                            in_=ob,
                        )

            if schedule == "staged":
                for st in range(SC):
                    qkv_stile(st, xc0 if st == 0 else None)
                    for pr in range(NPAIR):
                        attn(pr, st, st)
                for c in range(SC):
                    proj(c)
            else:
                raise ValueError(schedule)
    return nc


def _pbcast2(mask_sb):
    """mask [128, 128] -> [128, 2, 128] with broadcast middle dim."""
    m = mask_sb[:, :]
    ap = list(m.ap)
    return bass.AP(tensor=m.tensor, offset=m.offset, ap=[ap[0], [0, 2], ap[1]])


_NC_CACHE = {}


def _get_nc(nkt):
    if nkt not in _NC_CACHE:
        nc = bacc.Bacc(
            "TRN2",
            target_bir_lowering=False,
            debug=False,
            enable_asserts=False,
            num_devices=8,
        )
        _emit(nc, nkt)
        nc.compile()
        _NC_CACHE[nkt] = nc
    return _NC_CACHE[nkt]


def kernel(**inputs):
    x = np.asarray(inputs["x"], dtype=np.float32)
    mask = np.asarray(inputs["mask"]).reshape(S, S)
    Wq = np.asarray(inputs["Wq"], dtype=np.float32)
    bq = np.asarray(inputs["bq"], dtype=np.float32)
    Wk = np.asarray(inputs["Wk"], dtype=np.float32)
    bk = np.asarray(inputs["bk"], dtype=np.float32)
    Wv = np.asarray(inputs["Wv"], dtype=np.float32)
    bv = np.asarray(inputs["bv"], dtype=np.float32)
    Wo = np.asarray(inputs["Wo"], dtype=np.float32)
    bo = np.asarray(inputs["bo"], dtype=np.float32)

    assert np.array_equal(
        mask, np.tril(np.ones((S, S), dtype=bool))
    ), "kernel specialized for the causal (tril) mask"

    bias_zero = not (bq.any() or bk.any() or bv.any())
    nkt = 8 if bias_zero else 9
    nc = _get_nc(nkt)

    # local diag-block mask in (k, q) layout: valid when q >= k
    mtile = np.triu(np.ones((128, 128), dtype=np.float32)).astype(BF)

    def aug(w, b):
        if bias_zero:
            return w.astype(BF)
        pad = np.zeros((128, w.shape[1]), dtype=np.float32)
        pad[0] = b
        return np.vstack([w, pad]).astype(BF)

    in_maps = []
    for core in range(8):
        b, hg = divmod(core, 2)
        cols = slice(hg * DQK, (hg + 1) * DQK)
        xT = x[b].T
        if not bias_zero:
            pad = np.zeros((128, S), dtype=np.float32)
            pad[0] = 1.0
            xT = np.vstack([xT, pad])
        in_maps.append(
            {
                "xT": np.ascontiguousarray(xT).astype(BF),
                "wq": aug(Wq[:, cols], bq[cols]),
                "wk": aug(Wk[:, cols], bk[cols]),
                "wv": aug(Wv[:, cols], bv[cols]),
                "wo": np.ascontiguousarray(Wo[cols, :]).astype(BF),
                "mask": mtile,
            }
        )

    res = run_bass_kernel_spmd(nc, in_maps, core_ids=list(range(8)))
    global LAST_RESULT
    LAST_RESULT = res
    parts = [np.asarray(r["out"], dtype=np.float32) for r in res.results]
    out = np.stack(
        [parts[2 * b_] + parts[2 * b_ + 1] for b_ in range(B)]
    ) + bo.astype(np.float32)
    return out.astype(np.float32)

